# revision 60
# baseline (speedup 1.0000x reference)
"""Banded-DTW 1-NN (KnnDtw) Trainium2 Bass kernel — fwd/bwd split, fp16.

Algorithm
---------
Per (query q, fit row f): Sakoe-Chiba banded DTW (w=10, band j in
[i-10, i+10)) over length-256 sequences; output fit_labels[argmin_f dm].

Device mapping: two independent 127-step DP chains per pair,
  fwd:  rows 1..127 of the original DP,   band cell c<->j = i + c - 11
  bwd:  rows 254..128 as a forward DP on the reversed sequences,
        band cell c<->v = u + c - 10  (v = 255 - j; mirrored band)
stitched on host at the row 127/128 interface:
  dm = min_j [ min(F(j), F(j-1)) + B(j) ],  j in [118, 137].
Each chain step is 3 instructions: scalar-engine Abs (distance row),
vector tensor_tensor min (a[c] = min(prev[c], prev[c+1])), and one
tensor_tensor_scan (op0=min, op1=add) covering 32 pair-segments of
21 slots (guard + 20 cells); the guard's d = 2L resets the scan carry.
The two chains are independent, so the Tile scheduler interleaves them
and hides the per-instruction dependency-ack latency. Rows are fp16
(2x DVE mode for the tensor_tensor); the scan carry is fp32 internally.

fp16 rounding can perturb dm by up to ~1 absolute (observed 0.81), so the
host rechecks every query whose fp16 top-2 margin is within RECHECK_T by
recomputing the exact fp32 banded DTW (numpy) for the candidate fit rows
(~1-3% of pairs) and taking the exact argmin.
"""

import numpy as np

import bass_rust
import concourse.bass as bass
import concourse.bacc as bacc
import concourse.mybir as mybir
from concourse.tile import TileContext
from concourse import bass_utils

# Problem shapes (hardcoded per harness contract)
NQ, M = 128, 256      # samples
NF, N = 256, 256      # fit_data
W = 10
NCORES = 8
QPC = NQ // NCORES    # 16 queries per core
CELLS = 20            # band cells per row
SEG = CELLS + 1       # +1 guard slot that resets the scan carry
NSEG = 32             # segments (f_lo values) per partition
FD = NSEG * SEG       # 672 scan elements per partition per chain
PAD = 16              # fit row padding on each side
PADF = N + 2 * PAD    # 288
STEPS = M // 2        # 128 rows per half; 127 update steps per chain
L = np.float32(16384.0)   # exact in fp16; 2L = 32768 also exact
RECHECK_T = np.float32(2.5)
F32 = mybir.dt.float32
F16 = mybir.dt.float16

_CACHE: dict = {}


def _build_nc() -> bass.Bass:
    nc = bacc.Bacc(
        "TRN2", target_bir_lowering=False, debug=False, num_devices=NCORES
    )

    # fit is shipped as 4 column slices; the two small "head" slices cover the
    # first 29 steps of each chain and load in parallel on different engines,
    # so compute starts ~4us in instead of waiting for one 18KB DMA (~10us):
    #   t0a = padded cols [0:56)    (fwd steps i <= 29)
    #   t0b = padded cols [24:160)  (fwd steps i >= 30)
    #   t1a = padded cols [128:264) (bwd steps i >= 30, read reversed)
    #   t1b = padded cols [232:288) (bwd steps i <= 29, read reversed)
    t0a_in = nc.dram_tensor("fit_t0a", [128, NSEG * 56], F16, kind="ExternalInput")
    t0b_in = nc.dram_tensor("fit_t0b", [128, NSEG * 136], F16, kind="ExternalInput")
    t1a_in = nc.dram_tensor("fit_t1a", [128, NSEG * 136], F16, kind="ExternalInput")
    t1b_in = nc.dram_tensor("fit_t1b", [128, NSEG * 56], F16, kind="ExternalInput")
    nsamp_in = nc.dram_tensor("neg_samp", [128, M], F32, kind="ExternalInput")
    dtmpl_in = nc.dram_tensor("d_tmpl", [128, FD], F16, kind="ExternalInput")
    # step-1 operands precomputed on host (pure input prep): d1 = |fit - s_1|
    # distance rows, a1 = min(row0[c], row0[c+1]) shifted-min of the row-0
    # cumsum init. Shipping these lets step 1 skip its ACTs and TTs entirely;
    # row0 itself has no other on-device consumer.
    d1f_in = nc.dram_tensor("d1_f", [128, FD], F16, kind="ExternalInput")
    d1b_in = nc.dram_tensor("d1_b", [128, FD], F16, kind="ExternalInput")
    a1f_in = nc.dram_tensor("a1_f", [128, FD], F16, kind="ExternalInput")
    a1b_in = nc.dram_tensor("a1_b", [128, FD], F16, kind="ExternalInput")
    ff_out = nc.dram_tensor("ff_out", [128, FD], F16, kind="ExternalOutput")
    fb_out = nc.dram_tensor("fb_out", [128, FD], F16, kind="ExternalOutput")

    amin = mybir.AluOpType.min
    aadd = mybir.AluOpType.add
    fabs = mybir.ActivationFunctionType.Abs

    with TileContext(nc) as tc:
        with tc.tile_pool(name="main", bufs=1) as pool:
            t0a = pool.tile([128, NSEG * 56], F16)
            t0b = pool.tile([128, NSEG * 136], F16)
            t1a = pool.tile([128, NSEG * 136], F16)
            t1b = pool.tile([128, NSEG * 56], F16)
            nsamp = pool.tile([128, M], F32)
            rowf = [(pool.tile([128, FD + 1], F16, name=f"rowf{k}"), 0)
                    for k in range(2)]
            rowb = [(pool.tile([128, FD + 1], F16, name=f"rowb{k}"), 0)
                    for k in range(2)]

            def rsl(rb, lo, hi):
                t, b = rb
                return t[:, b + lo : b + hi]
            af = pool.tile([128, FD], F16)
            ab = pool.tile([128, FD], F16)
            df = [pool.tile([128, FD], F16, name=f"df{k}") for k in range(2)]
            db = [pool.tile([128, FD], F16, name=f"db{k}") for k in range(2)]

            # Each issuing engine (SP/ACT/GPSIMD) serializes its own DMAs, so
            # the gating loads (head fit slices, row0s, first d templates,
            # nsamp) go first and are spread across engines; the big tail fit
            # slices stream afterwards, overlapped with compute.
            nc.sync.dma_start(out=af[:], in_=a1f_in[:, :])
            nc.scalar.dma_start(out=ab[:], in_=a1b_in[:, :])
            nc.gpsimd.dma_start(out=df[1][:], in_=d1f_in[:, :])
            nc.gpsimd.dma_start(out=db[1][:], in_=d1b_in[:, :])
            nc.sync.dma_start(out=t0a[:], in_=t0a_in[:, :])
            nc.scalar.dma_start(out=t1b[:], in_=t1b_in[:, :])
            nc.gpsimd.dma_start(out=nsamp[:], in_=nsamp_in[:, :])
            nc.sync.dma_start(out=df[0][:], in_=dtmpl_in[:, :])
            nc.scalar.dma_start(out=db[0][:], in_=dtmpl_in[:, :])
            nc.scalar.dma_start(out=t0b[:], in_=t0b_in[:, :])
            nc.sync.dma_start(out=t1a[:], in_=t1a_in[:, :])
            # slot FD of every row buffer must read as +inf for the TT (the
            # scan only writes slots [0, FD))
            nc.vector.memset(rsl(rowf[0], FD, FD + 1), float(L))
            nc.vector.memset(rsl(rowf[1], FD, FD + 1), float(L))
            nc.vector.memset(rsl(rowb[0], FD, FD + 1), float(L))
            nc.vector.memset(rsl(rowb[1], FD, FD + 1), float(L))

            t0a3 = t0a.rearrange("p (s c) -> p s c", c=56)
            t0b3 = t0b.rearrange("p (s c) -> p s c", c=136)
            t1a3 = t1a.rearrange("p (s c) -> p s c", c=136)
            t1b3 = t1b.rearrange("p (s c) -> p s c", c=56)
            df3 = [d.rearrange("p (s c) -> p s c", c=SEG) for d in df]
            db3 = [d.rearrange("p (s c) -> p s c", c=SEG) for d in db]
            af3 = af.rearrange("p (s c) -> p s c", c=SEG)
            ab3 = ab.rearrange("p (s c) -> p s c", c=SEG)

            def shifted(view, delta):
                w = view.copy()
                w.offset = view.offset + delta
                return w

            def reversed_window(view, start_elem):
                # innermost [stride -1, count 20] starting at start_elem
                w = view.copy()
                ap = [list(p) for p in w.ap]
                ap[-1] = [-1, 20]
                w.ap = bass_rust.VecI64Pair(ap)
                w.offset = start_elem
                return w

            for i in range(1, STEPS):
                rfin, rfout = rowf[(i - 1) % 2], rowf[i % 2]
                rbin, rbout = rowb[(i - 1) % 2], rowb[i % 2]
                dfT, dbT = df[i % 2], db[i % 2]
                if i > 1:  # step-1 d rows are DMA-shipped precomputed
                    # fwd: d[c] = |fit[f, j] - s[q, i]|, j = i + c - 11, c in
                    # 1..20; padded window [i+6, i+26) from the covering slice
                    if i <= 29:
                        fsrc = t0a3[:, :, i + 6 : i + 26]
                    else:
                        fsrc = t0b3[:, :, i - 18 : i + 2]  # minus slice base 24
                    nc.scalar.activation(
                        out=df3[i % 2][:, :, 1 : SEG], in_=fsrc,
                        func=fabs, bias=nsamp[:, i : i + 1], scale=1.0,
                    )
                    # bwd: d[c] = |rf[f, v] - s[q, 255-i]|, v = i + c - 10;
                    # rf[v] = fit[255-v]: read reversed from padded col 280-i
                    if i <= 29:
                        bsrc = reversed_window(t1b3[:, :, 0:20], 48 - i)
                    else:
                        bsrc = reversed_window(t1a3[:, :, 0:20], 152 - i)
                    nc.scalar.activation(
                        out=db3[i % 2][:, :, 1 : SEG], in_=bsrc,
                        func=fabs, bias=nsamp[:, M - 1 - i : M - i], scale=1.0,
                    )
                if i > 1:  # step-1 a arrays are DMA-shipped precomputed
                    # a[c] = min(prev[c], prev[c+1]) over the cells per segment
                    rfin3 = rsl(rfin, 0, FD).rearrange("p (s c) -> p s c", c=SEG)
                    rbin3 = rsl(rbin, 0, FD).rearrange("p (s c) -> p s c", c=SEG)
                    nc.vector.tensor_tensor(
                        out=af3[:, :, 1:SEG], in0=rfin3[:, :, 1:SEG],
                        in1=shifted(rfin3[:, :, 1:SEG], 1), op=amin,
                    )
                    nc.vector.tensor_tensor(
                        out=ab3[:, :, 1:SEG], in0=rbin3[:, :, 1:SEG],
                        in1=shifted(rbin3[:, :, 1:SEG], 1), op=amin,
                    )
                nc.vector.tensor_tensor_scan(
                    out=rsl(rfout, 0, FD), data0=af[:, 0:FD], data1=dfT[:, 0:FD],
                    initial=float(L), op0=amin, op1=aadd,
                )
                nc.vector.tensor_tensor_scan(
                    out=rsl(rbout, 0, FD), data0=ab[:, 0:FD], data1=dbT[:, 0:FD],
                    initial=float(L), op0=amin, op1=aadd,
                )

            last = (STEPS - 1) % 2
            nc.sync.dma_start(out=ff_out[:, :], in_=rsl(rowf[last], 0, FD))
            nc.sync.dma_start(out=fb_out[:, :], in_=rsl(rowb[last], 0, FD))

    nc.compile()
    return nc


def _host_inputs(samples: np.ndarray, fit: np.ndarray):
    """Per-core in_maps for run_bass_kernel_spmd."""
    pidx = np.arange(128)
    fidx = (pidx % NCORES)[:, None] * NSEG + np.arange(NSEG)[None, :]  # [128,32]

    fit_pad = np.full((NF, PADF), L, np.float32)
    fit_pad[:, PAD : PAD + N] = fit
    fit_g = fit_pad[fidx].astype(np.float16)  # [128, 32, 288]

    def _slice(lo, hi):
        return np.ascontiguousarray(fit_g[:, :, lo:hi].reshape(128, -1))

    t0a, t0b = _slice(0, 56), _slice(24, 160)
    t1a, t1b = _slice(128, 264), _slice(232, 288)

    d_tmpl = np.full((128, NSEG, SEG), L, np.float16)
    d_tmpl[:, :, 0] = 2 * L
    d_tmpl = np.ascontiguousarray(d_tmpl.reshape(128, FD))
    fit_g32 = fit_g.astype(np.float32)

    in_maps = []
    for core in range(NCORES):
        qidx = core * QPC + pidx // NCORES  # [128]
        neg_samp = np.ascontiguousarray(-samples[qidx])

        # fwd row 0: cells c=11..20 <-> j=0..9: cumsum |s[q,0] - fit[f, 0..9]|
        row0f = np.full((128, NSEG, SEG), L, np.float32)
        d0 = np.abs(samples[qidx, 0][:, None, None] - fit[fidx][:, :, 0:10])
        row0f[:, :, 11:21] = np.cumsum(
            d0.astype(np.float16).astype(np.float32), axis=-1, dtype=np.float32)
        row0f = np.concatenate(
            [row0f.reshape(128, FD), np.full((128, 1), L, np.float32)], axis=1)

        # bwd row 0 (u=0): cells c=10..20 <-> v=0..10: cumsum |rs0 - rf(0..10)|
        row0b = np.full((128, NSEG, SEG), L, np.float32)
        rs0 = samples[qidx, M - 1][:, None, None]
        rfw = fit[fidx][:, :, ::-1][:, :, 0:11]
        d0b = np.abs(rs0 - rfw)
        row0b[:, :, 10:21] = np.cumsum(
            d0b.astype(np.float16).astype(np.float32), axis=-1, dtype=np.float32)
        row0b = np.concatenate(
            [row0b.reshape(128, FD), np.full((128, 1), L, np.float32)], axis=1)

        # step-1 d rows, matching the device ACT bit-for-bit (fp16 fit, fp32
        # abs, fp16 out): fwd reads padded cols [7, 27); bwd reads cols 280-c
        d1f = d_tmpl.reshape(128, NSEG, SEG).copy()
        d1f[:, :, 1:SEG] = np.abs(
            fit_g32[:, :, 7:27] - samples[qidx, 1][:, None, None]
        ).astype(np.float16)
        d1b = d_tmpl.reshape(128, NSEG, SEG).copy()
        d1b[:, :, 1:SEG] = np.abs(
            fit_g32[:, :, 279:259:-1] - samples[qidx, M - 2][:, None, None]
        ).astype(np.float16)

        # step-1 a arrays: shifted min of the fp16 row-0 state, guards = L
        # (bit-identical to what the device TT + guard memset would produce)
        def _a1(row0):
            r16 = row0.astype(np.float16)
            a = np.minimum(r16[:, 0:FD], r16[:, 1 : FD + 1])
            a.reshape(128, NSEG, SEG)[:, :, 0] = np.float16(L)
            return np.ascontiguousarray(a)

        a1f = _a1(row0f)
        a1b = _a1(row0b)

        in_maps.append(
            {
                "fit_t0a": t0a,
                "fit_t0b": t0b,
                "fit_t1a": t1a,
                "fit_t1b": t1b,
                "neg_samp": neg_samp,
                "d_tmpl": d_tmpl,
                "d1_f": np.ascontiguousarray(d1f.reshape(128, FD)),
                "d1_b": np.ascontiguousarray(d1b.reshape(128, FD)),
                "a1_f": a1f,
                "a1_b": a1b,
            }
        )
    return in_maps


def _assemble_dm(results) -> np.ndarray:
    """Stitch fwd/bwd final rows into dm [NQ, NF] (fp32, fp16-accuracy)."""
    dm = np.empty((NQ, NF), np.float32)
    jj = np.arange(118, 138)
    for core, res in enumerate(results):
        F = np.asarray(res["ff_out"], np.float16).astype(np.float32)
        B = np.asarray(res["fb_out"], np.float16).astype(np.float32)
        F = F.reshape(128, NSEG, SEG)
        B = B.reshape(128, NSEG, SEG)
        # F cells c=1..20 <-> j = c + 116; B cells c=1..20 <-> j = 138 - c
        Fj = np.full((128, NSEG, 141), np.float32(np.inf))
        Fj[:, :, 117:137] = F[:, :, 1:21]
        Bj = np.full((128, NSEG, 141), np.float32(np.inf))
        Bj[:, :, 118:138] = B[:, :, 20:0:-1]
        tot = np.minimum(Fj[:, :, jj], Fj[:, :, jj - 1]) + Bj[:, :, jj]
        d = tot.min(axis=2)  # [128, NSEG]
        d = d.reshape(QPC, NCORES, NSEG).reshape(QPC, NF)
        dm[core * QPC : (core + 1) * QPC] = d
    return dm


def _exact_dtw(samples_rows: np.ndarray, fit_rows: np.ndarray) -> np.ndarray:
    """Exact fp32 banded DTW (reference recurrence) for P (query,fit) pairs."""
    P, m = samples_rows.shape
    n = fit_rows.shape[1]
    INF = np.float32(np.inf)
    row = np.cumsum(np.abs(samples_rows[:, 0:1] - fit_rows), axis=1,
                    dtype=np.float32)
    for i in range(1, m):
        d_row = np.abs(samples_rows[:, i : i + 1] - fit_rows)
        new_col0 = row[:, 0] + d_row[:, 0]
        s = max(1, i - W)
        e = min(n, i + W)
        new_row = np.full((P, n), INF, np.float32)
        new_row[:, 0] = new_col0
        c = np.where(s == 1, new_col0, INF).astype(np.float32)
        for j in range(s, e):
            a = row[:, j] if j > 0 else INF
            a = np.minimum(row[:, j - 1], a)
            c = np.minimum(a, c) + d_row[:, j]
            new_row[:, j] = c
        row = new_row
    return row[:, -1]


def run_device(samples, fit, **spmd_kwargs):
    """Compile (cached) + run on 8 cores; returns (dm [128,256], results)."""
    if "nc" not in _CACHE:
        _CACHE["nc"] = _build_nc()
    nc = _CACHE["nc"]
    in_maps = _host_inputs(samples, fit)
    res = bass_utils.run_bass_kernel_spmd(
        nc, in_maps, core_ids=list(range(NCORES)), **spmd_kwargs
    )
    return _assemble_dm(res.results), res


def _labels_with_recheck(dm, samples, fit, labels):
    """argmin labels; exact fp32 recheck for queries with tight fp16 margins."""
    knn = np.argmin(dm, axis=1)
    mins = dm[np.arange(NQ), knn]
    cand_q, cand_f = np.nonzero(dm <= (mins[:, None] + RECHECK_T))
    multi = np.bincount(cand_q, minlength=NQ) > 1
    sel = multi[cand_q]
    cand_q, cand_f = cand_q[sel], cand_f[sel]
    if cand_q.size:
        exact = _exact_dtw(samples[cand_q], fit[cand_f])
        best = np.full(NQ, np.float32(np.inf))
        for k in range(cand_q.size):
            q = cand_q[k]
            if exact[k] < best[q]:
                best[q] = exact[k]
                knn[q] = cand_f[k]
    return labels[knn]


def kernel(samples, fit_data, fit_labels):
    samples = np.ascontiguousarray(np.asarray(samples), dtype=np.float32)
    fit = np.ascontiguousarray(np.asarray(fit_data), dtype=np.float32)
    labels = np.asarray(fit_labels)
    dm, _ = run_device(samples, fit)
    return _labels_with_recheck(dm, samples, fit, labels)
